# revision 37
# baseline (speedup 1.0000x reference)
"""Trainium2 Bass kernel for the FlowNet-style correlation layer.

Problem (hardcoded):
  x_1, x_2, p_1, p_2: [1, 64, 96, 96] f32;  img: [1, 1, 96, 96] f32
  x1 = concat(x_1, p_1) -> [1,128,96,96];  x2 = pad(concat(x_2,p_2), 20)
  out_vb[d, h, w]  = sum_c x1[c,h,w] * x2[c, h+dy, w+dx],  d = dy*41+dx
  out_img[d, h, w] = pad(img,20)[h+dy, w+dx]
  returns (out_vb [1,1681,96,96], out_img [1,1681,96,96])

Strategy: shard over output rows h (12 per core, 8 cores). Per core the
12 h-rows and a block of 10 w-columns are packed together into the
TensorE stationary dim (M = 120 x1 columns); the moving dim streams
(r, wp) pairs where r = h + dy is an absolute x2 row and wp an x2
column restricted to the block's reachable window
[max(0, w0-20), min(96, w0+wb+20)) — edge blocks stream no zero-pad
columns, the host masks out-of-range (w, dx) instead. 22048 moving
columns/core vs 47232 for the naive per-h Gram. A single bf16 pass
accumulates in fp32 PSUM (rel err ~3e-3, inside the 2e-2 gate).
Results drain via one f32->bf16 cast per 3-bank PSUM tile (amortizes
the ~400ns PSUM access latency; only DVE/Act may read PSUM) into a
compact per-block SBUF stage whose free axis is the contiguous (r, wp)
stream, stored with ONE DMA of M x ~4.7KB rows per block (DGE
descriptor cost is per row). The host casts back to f32 and extracts
the band out[dy, dx, h, w] = arr[h*wb+wl, h+dy, wl+dx-off] with a
strided view plus validity mask. x2 is loaded in r-chunks across two
queues so the first matmuls start ~3us in. out_img depends only on the
raw img input, so the host materializes it directly with stride
tricks; the device does no work for it.
"""

import numpy as np

import concourse.bass as bass
import concourse.tile as tile
from concourse import bacc, mybir
from concourse.bass_types import AP
from concourse.bass_utils import run_bass_kernel_spmd

F32 = mybir.dt.float32
BF16 = mybir.dt.bfloat16

H = W = 96
C2 = 128            # concat channels
PAD = 20
D = 2 * PAD + 1     # 41 displacements per axis
NCORES = 8
HS = H // NCORES    # 12 output rows per core
R = HS + 2 * PAD    # 52 absolute x2 rows per core
BANK = 512          # PSUM bank, f32 elems
NRC = 9             # r rows per matmul chunk (slot width 9*win <= 512)

# w-blocks: nine of 10 plus one of 6
WBS = [(10 * i, 10) for i in range(9)] + [(90, 6)]


def _block_geom(w0, wb, first=False):
    ws = max(0, w0 - PAD)
    we = min(W, w0 + wb + PAD)
    win = we - ws
    # as few chunks as fit the 512-f32 PSUM bank; uniform nr with a
    # (smaller) remainder LAST so slot-uniform casts put garbage past
    # the live stream only. The first processed block leads with a tiny
    # r<4 chunk so the PE starts as soon as the small first x2 load
    # chunk lands.
    if first:
        nch = 1 + -(-(R - 4) // (BANK // win))
        nr = -(-(R - 4) // (nch - 1))
        chunks = [(0, 4)]
        r0 = 4
    else:
        nch = -(-R // (BANK // win))
        nr = -(-R // nch)
        chunks = []
        r0 = 0
    while r0 < R:
        chunks.append((r0, min(nr, R - r0)))
        r0 += nr
    return ws, win, chunks


def _build_nc():
    nc = bacc.Bacc("TRN2", target_bir_lowering=False, debug=False,
                   num_devices=NCORES)

    # x1 permuted to [c, (block, h, wl)]; x2 h-padded rows [c, r, w]
    x1p = nc.declare_dram_parameter("x1p", [C2, HS * W], BF16, isOutput=False)
    x2p = nc.declare_dram_parameter("x2p", [C2, R * W], BF16, isOutput=False)
    CORR_SZ = sum(HS * wb * R * _block_geom(w0, wb)[1] for w0, wb in WBS)
    corr = nc.declare_dram_parameter("corr", [CORR_SZ], BF16, isOutput=True)

    with tile.TileContext(nc) as tc:
        with (
            tc.tile_pool(name="inp", bufs=1) as pin,
            tc.tile_pool(name="stage", bufs=6) as pst,
            tc.tile_pool(name="psum", bufs=4, space="PSUM") as pps,
        ):
            # vector/scalar are reserved for casts (any DMA issue on them
            # delays PSUM drains and stalls the PE); sync/gpsimd carry all
            # loads and stores. Loads are split so the first matmuls (x1
            # block 0, x2 r < 9) start as soon as the small first chunks
            # land.
            # the first processed block is 2 (x1 cols [240, 360)): load
            # that tiny lhs slice first so the first LDWEIGHTS isn't
            # gated on the full x1; first r-chunk rides the scalar
            # queue, which clears its preamble earliest
            x1_sb = pin.tile([C2, HS * W], BF16)
            nc.gpsimd.dma_start(x1_sb[:, :120], x1p[:, :120])
            nc.gpsimd.dma_start(x1_sb[:, 120:], x1p[:, 120:])
            # scalar's DMA ring is otherwise idle during the cast phase,
            # so it carries most load bytes; sync/gpsimd keep their
            # queues clear for the store stream
            x2_sb = pin.tile([C2, R * W], BF16)
            ld_engs = [nc.scalar, nc.scalar, nc.scalar, nc.sync, nc.gpsimd]
            for ci, (r0, r1) in enumerate([(0, 4), (4, 14), (14, 27),
                                           (27, 40), (40, 52)]):
                sl = slice(r0 * W, r1 * W)
                ld_engs[ci].dma_start(x2_sb[:, sl], x2p[:, sl])

            st_engs = [nc.sync, nc.gpsimd]
            cp_engs = [nc.vector, nc.scalar]
            ncast = 0
            # natural order: the small-N edge blocks burn through the
            # r-range slowly while the x2 load chunks stream in, and
            # the smallest block lands last (short drain tail)
            order = [0, 1, 2, 3, 4, 5, 6, 7, 8, 9]
            m_offs, c_offs = [], []
            mo = co = 0
            for w0, wb in WBS:
                m_offs.append(mo)
                c_offs.append(co)
                mo += HS * wb
                co += HS * wb * R * _block_geom(w0, wb)[1]
            nstore = 0
            late_stores = []
            for oi, bi in enumerate(order):
                w0, wb = WBS[bi]
                ws, win, chunks = _block_geom(w0, wb, first=(oi == 0))
                nch = len(chunks)
                M = HS * wb
                lhs = x1_sb[:, m_offs[bi]:m_offs[bi] + M]
                c_off = c_offs[bi]
                live = R * win
                # groups: the tiny lead chunk of the first block alone,
                # then pairs. Within a group the slot-uniform cast width
                # is the first chunk's width; a partial final chunk puts
                # garbage past the live stream, clamped out of the store.
                groups, g0 = [], 0
                if oi == 0:
                    groups.append([chunks[0]])
                    g0 = 1
                while g0 < nch:
                    groups.append(chunks[g0:g0 + 2])
                    g0 += 2
                sbw = sum(len(g) * g[0][1] * win for g in groups)
                sb = pst.tile([M, sbw], BF16)
                cum = 0
                for grp in groups:
                    ng = len(grp)
                    wmax = grp[0][1] * win
                    ps = pps.tile([M, 2 * BANK], F32)
                    for j, (r0, nr) in enumerate(grp):
                        x2ap = x2_sb[:]
                        rhs = AP(tensor=x2ap.tensor,
                                 offset=x2ap.offset + r0 * W + ws,
                                 ap=[[x2ap.ap[0][0], C2], [W, nr], [1, win]])
                        nc.tensor.matmul(ps[:, j * BANK: j * BANK + nr * win],
                                         lhs, rhs, start=True, stop=True)
                    psap = ps[:]
                    src = AP(tensor=psap.tensor, offset=psap.offset,
                             ap=[[psap.ap[0][0], M], [BANK, ng], [1, wmax]])
                    dst = sb[:, cum:cum + ng * wmax]
                    eng = cp_engs[ncast % len(cp_engs)]
                    ncast += 1
                    if eng is nc.scalar:
                        eng.copy(dst, src)
                    else:
                        eng.tensor_copy(dst, src)
                    cum += ng * wmax
                # one fat store per block (DGE descriptor-generation
                # cost is per row, so 120 rows x 5.2KB beats per-group
                # stores). The last three blocks store only half here;
                # the other halves are issued on scalar's queue after
                # its final cast, adding a third drain queue for the
                # end-of-kernel backlog.
                if oi >= len(order) - 3:
                    s0 = live // 2
                    d0 = AP(tensor=corr[:].tensor, offset=c_off,
                            ap=[[live, M], [1, s0]])
                    st_engs[nstore % 2].dma_start(d0, sb[:, :s0])
                    d1 = AP(tensor=corr[:].tensor, offset=c_off + s0,
                            ap=[[live, M], [1, live - s0]])
                    late_stores.append((d1, sb[:, s0:live]))
                else:
                    dstc = AP(tensor=corr[:].tensor, offset=c_off,
                              ap=[[live, M], [1, live]])
                    st_engs[nstore % 2].dma_start(dstc, sb[:, :live])
                nstore += 1
            for d1, src1 in late_stores:
                nc.scalar.dma_start(d1, src1)

    nc.compile()
    return nc


_NC_CACHE = None


def _get_nc():
    global _NC_CACHE
    if _NC_CACHE is None:
        _NC_CACHE = _build_nc()
    return _NC_CACHE


def _prep_in_maps(x_1, x_2, p_1, p_2):
    import ml_dtypes
    bf = ml_dtypes.bfloat16

    x1cat = np.concatenate([x_1[0], p_1[0]], axis=0).astype(bf)
    x2cat = np.concatenate([x_2[0], p_2[0]], axis=0).astype(bf)
    # h-pad 20 top/bottom with zeros; no w-padding (edge windows trimmed)
    x2pad = np.zeros((C2, H + 2 * PAD, W), bf)
    x2pad[:, PAD:PAD + H] = x2cat

    in_maps = []
    for i in range(NCORES):
        h0 = i * HS
        parts = []
        for w0, wb in WBS:
            blk = x1cat[:, h0:h0 + HS, w0:w0 + wb]  # [c, h, wl]
            parts.append(blk.reshape(C2, HS * wb))
        x1p = np.concatenate(parts, axis=1)
        in_maps.append({
            "x1p": np.ascontiguousarray(x1p),
            "x2p": np.ascontiguousarray(x2pad[:, h0:h0 + R]).reshape(
                C2, R * W),
        })
    return in_maps


def _vb_masks():
    """Per block: (off, mask[D, wb]) with mask=1 where wl+dx-off in
    [0, win) i.e. w+dx-PAD lands inside the streamed window (which is
    exactly the globally valid range for edge blocks)."""
    out = []
    for w0, wb in WBS:
        ws, win, _ = _block_geom(w0, wb)
        off = ws - (w0 - PAD)
        j = np.add.outer(np.arange(D), np.arange(wb)) - off  # [dx, wl]
        out.append((off, ((j >= 0) & (j < win)).astype(np.float32)))
    return out


_MASKS = _vb_masks()


def _postprocess(results, img):
    vb_parts = []
    for i in range(NCORES):
        corr = np.asarray(results[i]["corr"])
        vb = np.empty((D * D, HS, W), np.float32)
        c_off = 0
        for bi, (w0, wb) in enumerate(WBS):
            ws, win, _ = _block_geom(w0, wb)
            off, mask = _MASKS[bi]
            M = HS * wb
            arr = corr[c_off:c_off + M * R * win].astype(np.float32)
            # guard band so the strided view's out-of-window reads stay
            # inside the allocation (they are masked to zero anyway)
            buf = np.zeros(64 + M * R * win + 64, np.float32)
            buf[64:64 + M * R * win] = arr
            base = buf[64 - off:]
            # v[dy, dx, h, wl] = arr[h*wb+wl, h+dy, wl+dx-off]
            v = np.lib.stride_tricks.as_strided(
                base, shape=(D, D, HS, wb),
                strides=(4 * win, 4, 4 * (wb * R * win + win),
                         4 * (R * win + 1)))
            vb[:, :, w0:w0 + wb] = (v * mask[None, :, None, :]).reshape(
                D * D, HS, wb)
            c_off += M * R * win
        vb_parts.append(vb)
    out_vb = np.concatenate(vb_parts, axis=1)[None]

    imgp = np.zeros((H + 2 * PAD, W + 2 * PAD), np.float32)
    imgp[PAD:PAD + H, PAD:PAD + W] = img[0, 0]
    si = imgp.strides
    iv = np.lib.stride_tricks.as_strided(
        imgp, shape=(D, D, H, W), strides=(si[0], si[1], si[0], si[1]))
    out_img = np.ascontiguousarray(iv).reshape(1, D * D, H, W)
    return out_vb, out_img


def kernel(x_1, x_2, img, p_1, p_2, _trace=False):
    nc = _get_nc()
    in_maps = _prep_in_maps(np.asarray(x_1), np.asarray(x_2),
                            np.asarray(p_1), np.asarray(p_2))
    res = run_bass_kernel_spmd(nc, in_maps, list(range(NCORES)), trace=_trace)
    out = _postprocess(res.results, np.asarray(img))
    if _trace:
        return out, res
    return out
